# revision 1
# baseline (speedup 1.0000x reference)
"""Trainium2 Bass kernel for nn_MultiHeadModel (segment_reduce), 8-core SPMD.

Reference math:
    xp  = x @ Wp + bp                              # [N, 256]
    class_emb[g] = (sum_{i in g} m_i * xp_i) / n_g # [G, 256]  (segment mean)
    h   = concat(repeat(class_emb, C), xp[idx])    # [G*C, 512]
    out = relu(relu(h@W1+b1)@W2+b2) @ W3 + b3      # [G*C, 1]
(edge_attr's projection is dead code - never touched.)

Sharding: data-parallel over graphs, 128 graphs + their masked nodes + their
2048 output rows per core; weights replicated; no collectives.

HW-measured design points:
  *  The PE streams 1 rhs column/cycle for fp16 (fp8 DoubleRow streams the
     doubled free dim, i.e. 2 fp8 cols/cycle when back-to-back). The MLP row
     path (h1/h2/head) stays fp16 - fp8 anywhere in it fails the 2e-2 gate
     (measured 4-8e-2). fp8 is used where it cuts DMA or instructions:
       - segment stream in fp8 with HOST-PREBUILT 0/1 indicator blocks
         packed per tile-pair ([xA 256|xB 256|indA 32|indB 32] B/partition);
         one DR matmul per pair, no on-device indicator build, half the
         stream DMA bytes of the fp16 baseline.
       - cls chain fp8-DR (Wt pre-scaled x32, class-means x16; the 1/512
         unscale is free in the cls1b copy's ACTIVATE scale).
  *  cbias folds into the h1 relu bias; b2 into the h2 relu bias; b3 is
     added host-side. GpSimd computes nothing (no PSUM access on TRN2 and
     ~9us/op measured for bulk tensor_scalar).
  *  Engine split: vector = cls scales + sxT + h1 adds (in-place in PSUM)
     + out copies; scalar = h1/h2 relus + cls1b unscale-copy.
  *  DMA: consts split A/B1/B2 so the h1-critical prefix (wb+xg0) and the
     cls consts land first, 4 big fp8 stream DMAs, one tiny wt8 DMA on the
     scalar queue, one output DMA.  Quarter q's cls chain is emitted at its
     last pair; finish(q) (adds, relus, h2, head) 2 pairs later so it
     executes during the remaining stream; only quarter 3 forms a tail.
"""
import numpy as np
import ml_dtypes
from contextlib import ExitStack

import concourse.bacc as bacc
import concourse.mybir as mybir
from concourse.tile import TileContext
from concourse.bass_utils import run_bass_kernel_spmd

M = 8                 # cores
G = 1024              # graphs
C = 16                # classes
GL = G // M           # graphs per core (128)
D = 256
ROWS = G * C // M     # MLP rows per core (2048)
NCH = 4               # 512-row chunks == quarters
SC1 = 32.0            # Wt pre-scale (fp8 mantissa headroom)
SC2 = 16.0            # class-mean pre-scale
PW = 576              # stream bytes/partition per tile-pair
SPAIRS = 8            # tile-pairs per stream DMA (16 node tiles)

f32 = mybir.dt.float32
f16 = mybir.dt.float16
f8 = mybir.dt.float8e4
np8 = ml_dtypes.float8_e4m3
Relu = mybir.ActivationFunctionType.Relu
Copy = mybir.ActivationFunctionType.Copy
DR = mybir.MatmulPerfMode.DoubleRow
ADD = mybir.AluOpType.add

# ---- fp16 const tensor column offsets (f32 fields at even offsets) ----
WBO = 0
XGO = {0: 1024, 1: 2140, 2: 4188, 3: 5212}
INVO, IDO, CBO, B2O, B3O, W3O = 2048, 2056, 2120, 2128, 2132, 2134
W2O = 3164
C16W = 6236
CUT_A, CUT_B = 2140, 4188
C8W = 1024            # wt8 only

_cache = {}


def _build(BQ):
    """BQ[q] = node-tile count of quarter q (even). Quarter q holds graphs
    [32q, 32q+32) and output-row chunk q."""
    NT = sum(BQ)
    NP = NT // 2
    BP = [b // 2 for b in BQ]
    pq_end = np.cumsum(BP)
    NS = (NP + SPAIRS - 1) // SPAIRS

    nc = bacc.Bacc(None, target_bir_lowering=False, debug=False)
    xstr = nc.dram_tensor("xstr", [128, NP * PW], f8, kind="ExternalInput")
    cpk8 = nc.dram_tensor("cpk8", [128, C8W], f8, kind="ExternalInput")
    cpk16 = nc.dram_tensor("cpk16", [128, C16W], f16, kind="ExternalInput")
    out = nc.dram_tensor("out", [1, ROWS], f32, kind="ExternalOutput")

    with TileContext(nc) as tc, ExitStack() as ctx:
        cst = ctx.enter_context(tc.tile_pool(name="cst", bufs=1))
        stream = ctx.enter_context(tc.tile_pool(name="stream", bufs=4))
        pseg = ctx.enter_context(tc.tile_pool(name="pseg", bufs=1, space="PSUM"))
        pml = ctx.enter_context(tc.tile_pool(name="pml", bufs=4, space="PSUM"))
        pcls = ctx.enter_context(tc.tile_pool(name="pcls", bufs=1, space="PSUM"))

        c8 = cst.tile([128, C8W], f8, tag="c8")
        c16 = cst.tile([128, C16W], f16, tag="c16")

        # --- DMA issue order (sync): h1-critical prefix + cls consts,
        # --- stream 0, xg1+w2, stream 1, xg2/3, streams 2+.  wt8 on scalar.
        # split so the first h1 matmul only gates on wb + xg0-k0 (384KB)
        nc.sync.dma_start(out=c16[:, :1536], in_=cpk16[:, :1536])
        nc.sync.dma_start(out=c16[:, 1536:CUT_A], in_=cpk16[:, 1536:CUT_A])

        st_tiles = []

        def stream_dma(i):
            p0 = i * SPAIRS
            npr = min(SPAIRS, NP - p0)
            t = stream.tile([128, SPAIRS * PW], f8, tag="s")
            nc.sync.dma_start(out=t[:, :npr * PW],
                              in_=xstr[:, p0 * PW:(p0 + npr) * PW])
            return t

        st_tiles.append(stream_dma(0))
        nc.scalar.dma_start(out=c8[:], in_=cpk8[:])
        nc.sync.dma_start(out=c16[:, CUT_A:CUT_B], in_=cpk16[:, CUT_A:CUT_B])
        if NS > 1:
            st_tiles.append(stream_dma(1))
        nc.sync.dma_start(out=c16[:, CUT_B:C16W], in_=cpk16[:, CUT_B:C16W])
        for i in range(2, NS):
            st_tiles.append(stream_dma(i))

        # --- const views ---
        def wb_ap(k2):
            return c16[:, WBO + 512 * k2:WBO + 512 * (k2 + 1)]

        def xg_ap(n, k2):
            return c16[:, XGO[n] + 512 * k2:XGO[n] + 512 * (k2 + 1)]
        w2v = c16[:, W2O:W2O + 1024].rearrange("p (k m) -> p k m", k=4)
        w3v = c16[:, W3O:W3O + 4].rearrange("p (k m) -> p k m", k=2)
        cbv = c16[:, CBO:CBO + 8].bitcast(f32)        # [128, 4]
        b2v = c16[:, B2O:B2O + 4].bitcast(f32)        # [128, 2]
        invv = c16[:, INVO:INVO + 8].bitcast(f32)     # [128, 4]
        identv = c16[:, IDO:IDO + 64].bitcast(f32)    # [128, 32]
        wtv = c8[:, 0:1024].rearrange("p (two m) -> p two m", two=2)

        psQ = pseg.tile([32, 4, 256], f32, tag="psQ")
        osb = cst.tile([1, ROWS], f32, tag="osb")
        h1psum = [[None] * NCH for _ in range(4)]
        cls1b = {}

        def h1job(m1, n):
            ph = pml.tile([128, 512], f32, tag="mlp", name=f"ph{m1}{n}")
            for k2 in range(2):
                nc.tensor.matmul(out=ph[:],
                                 lhsT=wb_ap(k2)[:, m1 * 128:(m1 + 1) * 128],
                                 rhs=xg_ap(n, k2),
                                 start=(k2 == 0), stop=(k2 == 1))
            h1psum[m1][n] = ph

        def cls_chain(q):
            sxs = cst.tile([32, 256], f32, tag=f"sxs{q}")
            nc.vector.tensor_scalar_mul(out=sxs[:], in0=psQ[:32, q, :],
                                        scalar1=invv[:32, q:q + 1])
            clsT = pcls.tile([128, 192], f32, tag="clsT", name=f"clsT{q}")
            psT = clsT[:, 128:192]
            clsP = clsT[:, 0:128].rearrange("p (m g) -> p m g", m=4)
            for c2 in range(2):
                nc.tensor.transpose(out=psT[:, c2 * 32:(c2 + 1) * 32],
                                    in_=sxs[:, c2 * 128:(c2 + 1) * 128],
                                    identity=identv[:32, :])
            sxT = cst.tile([128, 2, 32], f8, tag=f"sxT{q}")
            nc.vector.tensor_scalar_mul(
                out=sxT[:].rearrange("p a b -> p (a b)"), in0=psT[:],
                scalar1=SC2)
            for m1 in range(4):
                nc.tensor.matmul(out=clsP[:, m1, :],
                                 lhsT=wtv[:, :, m1 * 128:(m1 + 1) * 128],
                                 rhs=sxT[:], perf_mode=DR, start=True, stop=True)
            cb16 = cst.tile([128, 4, 32], f16, tag=f"cls1b{q}")
            nc.scalar.activation(out=cb16[:].rearrange("p a b -> p (a b)"),
                                 in_=clsT[:, 0:128], func=Copy,
                                 scale=1.0 / (SC1 * SC2))
            cls1b[q] = cb16

        def finish(n):
            h1ts = []
            for m1 in range(4):
                ph = h1psum[m1][n]
                ph3 = ph[:].rearrange("p (g c) -> p g c", c=C)
                nc.vector.tensor_tensor(
                    out=ph3, in0=ph3,
                    in1=cls1b[n][:, m1, :, None].to_broadcast([128, 32, C]),
                    op=ADD)
                h1t = cst.tile([128, 512], f16, tag=f"h1t{m1}{n}")
                nc.scalar.activation(out=h1t[:], in_=ph[:], func=Relu,
                                     bias=cbv[:, m1:m1 + 1])
                h1ts.append(h1t)
            h2ts = []
            for m2 in range(2):
                ph2 = pml.tile([128, 512], f32, tag="mlp", name=f"ph2{m2}{n}")
                for k4 in range(4):
                    nc.tensor.matmul(out=ph2[:],
                                     lhsT=w2v[:, k4, m2 * 128:(m2 + 1) * 128],
                                     rhs=h1ts[k4][:],
                                     start=(k4 == 0), stop=(k4 == 3))
                h2t = cst.tile([128, 512], f16, tag=f"h2t{m2}{n}")
                nc.scalar.activation(out=h2t[:], in_=ph2[:], func=Relu,
                                     bias=b2v[:, m2:m2 + 1])
                h2ts.append(h2t)
            po = pml.tile([1, 512], f32, tag="mlp", name=f"po{n}")
            for m2 in range(2):
                nc.tensor.matmul(out=po[:1, :], lhsT=w3v[:, m2, 0:1],
                                 rhs=h2ts[m2][:],
                                 start=(m2 == 0), stop=(m2 == 1))
            # b3 is added host-side after the gather
            if n == NCH - 1:
                nc.scalar.activation(out=osb[:1, n * 512:(n + 1) * 512],
                                     in_=po[:1, :], func=Copy)
            else:
                nc.vector.tensor_copy(out=osb[:1, n * 512:(n + 1) * 512],
                                      in_=po[:1, :])
            nc.sync.dma_start(out=out[:1, n * 512:(n + 1) * 512],
                              in_=osb[:1, n * 512:(n + 1) * 512])

        # --- schedule ---
        unlocked = [(m1, 0) for m1 in range(4)]
        pending = []
        h1job(*unlocked.pop(0))
        h1job(*unlocked.pop(0))
        q = 0
        for p in range(NP):
            stile = st_tiles[p // SPAIRS]
            j = p % SPAIRS
            xpair = stile[:, j * PW:j * PW + 512].rearrange(
                "p (two f) -> p two f", two=2)
            ipair = stile[:, j * PW + 512:j * PW + 576].rearrange(
                "p (two f) -> p two f", two=2)
            qlo = pq_end[q - 1] if q else 0
            nc.tensor.matmul(out=psQ[:32, q, :], lhsT=ipair, rhs=xpair,
                             perf_mode=DR, start=(p == qlo),
                             stop=(p == pq_end[q] - 1))
            if pending and p >= pending[0][0]:
                _, fq = pending.pop(0)
                for job in [jb for jb in unlocked if jb[1] == fq]:
                    unlocked.remove(job)
                    h1job(*job)
                finish(fq)
                if fq + 1 < NCH:
                    unlocked.extend((m1, fq + 1) for m1 in range(4))
            if unlocked:
                h1job(*unlocked.pop(0))
            if p == pq_end[q] - 1:
                cls_chain(q)
                pending.append((p + 2, q))
                q += 1
        for _, fq in pending:
            for job in [jb for jb in unlocked if jb[1] == fq]:
                unlocked.remove(job)
                h1job(*job)
            finish(fq)
        for job in unlocked:
            h1job(*job)

    nc.compile()
    return nc


def _pack_consts(Wb, wt8, w2s, W3, cbias, b2, b3, invk, xgt):
    c8 = np.zeros((128, C8W), np8)
    c8[:, :1024] = wt8.reshape(2, 128, 512).transpose(1, 0, 2).reshape(128, 1024)

    c16 = np.zeros((128, C16W), np.float16)
    wb16 = Wb.astype(np.float16)
    xg16 = xgt.astype(np.float16)
    for k2 in range(2):
        c16[:, WBO + k2 * 512:WBO + (k2 + 1) * 512] = wb16[k2 * 128:(k2 + 1) * 128]
    for n in range(NCH):
        for k2 in range(2):
            c16[:, XGO[n] + k2 * 512:XGO[n] + (k2 + 1) * 512] = \
                xg16[k2 * 128:(k2 + 1) * 128, n * 512:(n + 1) * 512]
    c16[:, W2O:W2O + 1024] = w2s.astype(np.float16).reshape(
        4, 128, 256).transpose(1, 0, 2).reshape(128, 1024)
    c16[:, W3O:W3O + 4:2] = W3.astype(np.float16).reshape(2, 128).T

    def put32(off, arr):
        a = np.ascontiguousarray(arr, np.float32).view(np.float16)
        c16[:a.shape[0], off:off + a.shape[1]] = a
    put32(CBO, cbias.reshape(4, 128).T)
    put32(B2O, b2.reshape(2, 128).T)
    put32(B3O, np.zeros((1, 1), np.float32))
    put32(INVO, invk.reshape(4, 32).T)
    put32(IDO, np.eye(32, dtype=np.float32))
    return np.ascontiguousarray(c8), np.ascontiguousarray(c16)


def kernel(x, edge_attr, batch, target_node_mask, true_nodes_idx,
           Wp, bp, W1, b1, W2, b2, W3, b3,
           num_graphs=G, num_classes=C, **_):
    x = np.ascontiguousarray(np.asarray(x), dtype=np.float32)
    batch = np.asarray(batch).astype(np.int64)
    mask = np.asarray(target_node_mask).astype(bool)
    idx = np.asarray(true_nodes_idx).astype(np.int64)
    Wp = np.asarray(Wp, np.float32)
    W1 = np.asarray(W1, np.float32)
    W2 = np.ascontiguousarray(np.asarray(W2), np.float32)
    W3 = np.ascontiguousarray(np.asarray(W3), np.float32)
    bp = np.asarray(bp, np.float32)
    b1 = np.asarray(b1, np.float32)
    b2 = np.asarray(b2, np.float32)
    b3 = np.asarray(b3, np.float32)

    Wt = (Wp @ W1[:D]).astype(np.float32)          # [256, 512]
    Wb = (Wp @ W1[D:]).astype(np.float32)          # [256, 512]
    cbias = (bp @ (W1[:D] + W1[D:]) + b1).astype(np.float32)
    wt8 = (Wt * SC1).astype(np8)

    ncount = np.bincount(batch[mask], minlength=G).astype(np.float32)
    with np.errstate(divide="ignore"):
        inv_all = (np.float32(1.0) / ncount).astype(np.float32)

    core = batch // GL
    quarter = (batch % GL) // 32
    selq = [[np.flatnonzero((core == k) & mask & (quarter == qq))
             for qq in range(4)] for k in range(M)]
    BQ = []
    for qq in range(4):
        t = max(1, max((len(selq[k][qq]) + 127) // 128 for k in range(M)))
        BQ.append(t + (t & 1))
    BQ = tuple(BQ)
    NT = sum(BQ)
    NP = NT // 2

    if BQ not in _cache:
        _cache[BQ] = _build(BQ)
    nc = _cache[BQ]

    in_maps = []
    for k in range(M):
        Xt = np.zeros((NT * 128, D), np8)
        It = np.zeros((NT * 128, 32), np8)
        lo = 0
        for qq in range(4):
            rows = selq[k][qq]
            nk = len(rows)
            Xt[lo:lo + nk] = x[rows].astype(np8)
            It[lo + np.arange(nk), batch[rows] - k * GL - 32 * qq] = 1.0
            lo += BQ[qq] * 128
        Xp = Xt.reshape(NP, 2, 128, D).transpose(2, 0, 1, 3).reshape(128, NP, 512)
        Ip = It.reshape(NP, 2, 128, 32).transpose(2, 0, 1, 3).reshape(128, NP, 64)
        xstr = np.ascontiguousarray(
            np.concatenate([Xp, Ip], axis=2).reshape(128, NP * PW))

        xgt = np.ascontiguousarray(x[idx[k * ROWS:(k + 1) * ROWS]].T)
        invk = inv_all[k * GL:(k + 1) * GL]
        c8a, c16a = _pack_consts(Wb, wt8, W2, W3, cbias, b2, b3, invk, xgt)
        in_maps.append(dict(xstr=xstr, cpk8=c8a, cpk16=c16a))

    res = run_bass_kernel_spmd(nc, in_maps, list(range(M)))
    out = np.concatenate([res.results[k]["out"].reshape(ROWS) for k in range(M)])
    return (out + b3[0]).reshape(G * C, 1).astype(np.float32)

